# revision 1
# baseline (speedup 1.0000x reference)
"""Trainium2 Bass kernel for pooled cross-attention block (dense_transformer).

Reference computation per batch element b (B=8, one per NeuronCore):
  x2p = 2x2 mean-pool(x2)                      [512, 32, 32]
  Q = Wq @ x1  + bq                            [64, 4096]   (d-part layout)
  K = Wk @ x2p + bk                            [64, 1024]
  V = Wv @ x2p + bv                            [64, 1024]
  attn = softmax_n(Q^T K)                      [4096, 1024]
  out  = attn @ V^T                            [4096, 64]
  y    = out @ Wo^T + bo -> [256, 4096] ; result = x1 + y

Kernel strategy (all on-chip per core, streamed over n in 512-col chunks):
  - scores computed TRANSPOSED: sT[m, n] = K^T Q so softmax's reduce dim m
    is the partition dim; the row-sum r[n] is obtained for free by
    augmenting V^T with a ones column (row 64 of the U = V_aug^T expS
    accumulation).  No PE transposes anywhere.
  - bias algebra (all exact): bk drops (per-row softmax shift invariance);
    bq folded into Q via per-partition bias on the PSUM->SBUF copy;
    bv folded into bo' = bo + Wo@bv on host (attn rows sum to 1);
    bo' enters via the ones-row of the normalized U (row 64 == 1 after
    dividing by r) against an augmented Wo^T.
  - 2x2 pooling: two strided DVE adds; the 1/4 scale is folded into Wk/Wv
    on the host.
  - dtypes: Q projection runs float32r on raw fp32 x1 (full PE rate at
    N=512, no cast cost); everything downstream of the projections is
    bf16 on the PE with fp32 PSUM accumulation.
"""

import sys

for _p in ("/opt/trn_rl_repo",):
    if _p not in sys.path:
        sys.path.insert(0, _p)

import numpy as np

B, C1, C2, H, W, D = 8, 256, 512, 64, 64, 64
HW = H * W            # n (query) size: 4096
M = (H // 2) * (W // 2)  # kv size: 1024
NCH = 512             # n-chunk (one fp32 PSUM bank)
NCHUNKS = HW // NCH   # 8
C1T = C1 // 128       # 2
C2T = C2 // 128       # 4
MT = M // 128         # 8

_CACHE = {}


def _build():
    import concourse.bass as bass
    import concourse.tile as tile
    from concourse import bacc, mybir

    dt = mybir.dt
    f32, bf16, f32r = dt.float32, dt.bfloat16, dt.float32r
    Exp = mybir.ActivationFunctionType.Exp

    nc = bacc.Bacc(
        "TRN2", target_bir_lowering=False, debug=False, num_devices=8
    )
    x1 = nc.dram_tensor("x1", [C1, HW], f32r, kind="ExternalInput").ap()
    x2 = nc.dram_tensor("x2", [C2, HW], f32, kind="ExternalInput").ap()
    wqt = nc.dram_tensor("wqt", [C1T, 128, D], f32r, kind="ExternalInput").ap()
    wkt = nc.dram_tensor("wkt", [C2T, 128, D], bf16, kind="ExternalInput").ap()
    wvt = nc.dram_tensor("wvt", [C2T, 128, D], bf16, kind="ExternalInput").ap()
    wot = nc.dram_tensor("wot", [D + 1, C1], bf16, kind="ExternalInput").ap()
    bq = nc.dram_tensor("bq", [D, 1], f32, kind="ExternalInput").ap()
    out = nc.dram_tensor("out", [C1, HW], f32, kind="ExternalOutput").ap()

    from contextlib import ExitStack

    with tile.TileContext(nc) as tc, ExitStack() as ctx:
        pool = lambda name, bufs, **kw: ctx.enter_context(
            tc.tile_pool(name=name, bufs=bufs, **kw)
        )
        consts = pool("consts", 1)
        x2p = pool("x2p", 8)
        poolp = pool("poolp", 8)
        sp = pool("sp", 8)
        kvsb = pool("kvsb", 1)
        x1p = pool("x1p", 8)
        qsbp = pool("qsbp", 2)
        esp = pool("esp", 3)
        rp = pool("rp", 2)
        onp = pool("onp", 2)
        youtp = pool("youtp", 6)
        ps_a = pool("ps_a", 2, space="PSUM")
        ps_b = pool("ps_b", 2, space="PSUM")
        ps_s = pool("ps_s", 2, space="PSUM")
        ps_y = pool("ps_y", 2, space="PSUM")

        # ---- constants -------------------------------------------------
        wqt_sb = consts.tile([128, C1T, D], f32r, tag="wqt")
        for t in range(C1T):
            nc.sync.dma_start(out=wqt_sb[:, t, :], in_=wqt[t])
        wkt_sb = consts.tile([128, C2T, D], bf16, tag="wkt")
        wvt_sb = consts.tile([128, C2T, D], bf16, tag="wvt")
        for t in range(C2T):
            nc.sync.dma_start(out=wkt_sb[:, t, :], in_=wkt[t])
            nc.sync.dma_start(out=wvt_sb[:, t, :], in_=wvt[t])
        wot_sb = consts.tile([D + 1, C1], bf16, tag="wot")
        nc.sync.dma_start(out=wot_sb, in_=wot)
        bq_sb = consts.tile([D, 1], f32, tag="bq")
        nc.sync.dma_start(out=bq_sb, in_=bq)

        # ---- phase A: pool x2, project K and V^T -----------------------
        k_ps = [ps_a.tile([D, NCH], f32, tag="kq", name=f"k_ps{h}") for h in range(2)]
        v_ps = ps_b.tile([128, MT, D], f32, tag="uv", name="v_ps")
        half = HW // 2
        for ci in range(C2T):
            for hi in range(2):
                x2t = x2p.tile([128, half], f32, tag="x2t", name="x2t")
                nc.sync.dma_start(
                    out=x2t,
                    in_=x2[ci * 128:(ci + 1) * 128, hi * half:(hi + 1) * half],
                )
                x2v = x2t.rearrange("p (h w2 two) -> p h w2 two", w2=W // 2, two=2)
                t1 = poolp.tile([128, H // 2, W // 2], f32, tag="t1", name="t1")
                nc.vector.tensor_add(t1, x2v[:, :, :, 0], x2v[:, :, :, 1])
                t1v = t1.rearrange("p (h2 two) w2 -> p h2 two w2", two=2)
                s_bf = sp.tile([128, NCH], bf16, tag="s", name="s_bf")
                s3 = s_bf.rearrange("p (h2 w2) -> p h2 w2", h2=H // 4)
                nc.vector.tensor_add(s3, t1v[:, :, 0, :], t1v[:, :, 1, :])
                nc.tensor.matmul(
                    k_ps[hi],
                    lhsT=wkt_sb[:, ci, :],
                    rhs=s_bf,
                    start=(ci == 0),
                    stop=(ci == C2T - 1),
                )
                for mj in range(MT // 2):
                    mi = hi * (MT // 2) + mj
                    nc.tensor.matmul(
                        v_ps[:, mi, :],
                        lhsT=s_bf[:, mj * 128:(mj + 1) * 128],
                        rhs=wvt_sb[:, ci, :],
                        start=(ci == 0),
                        stop=(ci == C2T - 1),
                    )
        k_sb = kvsb.tile([D, M], bf16, tag="ksb")
        for h in range(2):
            nc.vector.tensor_copy(k_sb[:, h * NCH:(h + 1) * NCH], k_ps[h])
        v_aug = kvsb.tile([128, MT, D + 1], bf16, tag="vaug")
        nc.vector.memset(v_aug[:, :, D], 1.0)
        for mi in range(MT):
            nc.vector.tensor_copy(v_aug[:, mi, 0:D], v_ps[:, mi, :])

        # ---- phase B: stream n-chunks ----------------------------------
        for nj in range(NCHUNKS):
            nsl = slice(nj * NCH, (nj + 1) * NCH)
            x1t = x1p.tile([128, C1T, NCH], f32r, tag="x1t", name="x1t")
            for t in range(C1T):
                nc.sync.dma_start(out=x1t[:, t, :], in_=x1[t * 128:(t + 1) * 128, nsl])
            q_ps = ps_a.tile([D, NCH], f32, tag="kq", name="q_ps")
            for t in range(C1T):
                nc.tensor.matmul(
                    q_ps,
                    lhsT=wqt_sb[:, t, :],
                    rhs=x1t[:, t, :],
                    start=(t == 0),
                    stop=(t == C1T - 1),
                )
            q_sb = qsbp.tile([D, NCH], bf16, tag="qsb", name="q_sb")
            nc.scalar.add(q_sb, q_ps, bq_sb)

            u_ps = ps_b.tile([D + 1, NCH], f32, tag="uv", name="u_ps")
            for mi in range(MT):
                s_ps = ps_s.tile([128, NCH], f32, tag="st", name="s_ps")
                nc.tensor.matmul(
                    s_ps,
                    lhsT=k_sb[:, mi * 128:(mi + 1) * 128],
                    rhs=q_sb,
                    start=True,
                    stop=True,
                )
                es = esp.tile([128, NCH], bf16, tag="es", name="es")
                nc.scalar.activation(es, s_ps, Exp)
                nc.tensor.matmul(
                    u_ps,
                    lhsT=v_aug[:, mi, :],
                    rhs=es,
                    start=(mi == 0),
                    stop=(mi == MT - 1),
                )
            rinv = rp.tile([1, NCH], f32, tag="rinv", name="rinv")
            nc.vector.reciprocal(rinv, u_ps[D:D + 1, :])
            rb = rp.tile([D + 1, NCH], f32, tag="rb", name="rb")
            nc.gpsimd.partition_broadcast(rb, rinv)
            on = onp.tile([D + 1, NCH], bf16, tag="on", name="on")
            nc.vector.tensor_mul(on, u_ps, rb)
            for t in range(C1T):
                y_ps = ps_y.tile([128, NCH], f32, tag="y", name="y_ps")
                nc.tensor.matmul(
                    y_ps,
                    lhsT=wot_sb[:, t * 128:(t + 1) * 128],
                    rhs=on,
                    start=True,
                    stop=True,
                )
                yo = youtp.tile([128, NCH], f32, tag="yo", name="yo")
                nc.vector.tensor_add(yo, x1t[:, t, :].bitcast(f32), y_ps)
                nc.sync.dma_start(out=out[t * 128:(t + 1) * 128, nsl], in_=yo)
    nc.compile()
    return nc


def _get_nc():
    if "nc" not in _CACHE:
        _CACHE["nc"] = _build()
    return _CACHE["nc"]


def _prep_in_maps(x1, x2, Wq, bq, Wk, bk, Wv, bv, Wo, bo):
    import ml_dtypes

    bf16 = ml_dtypes.bfloat16
    f32 = np.float32
    x1 = np.asarray(x1, f32)
    x2 = np.asarray(x2, f32)
    Wq = np.asarray(Wq, f32)
    Wk = np.asarray(Wk, f32)
    Wv = np.asarray(Wv, f32)
    Wo = np.asarray(Wo, f32)
    bq = np.asarray(bq, f32)
    bk = np.asarray(bk, f32)
    bv = np.asarray(bv, f32)
    bo = np.asarray(bo, f32)

    wqt = np.ascontiguousarray(Wq.T.reshape(C1T, 128, D))
    wkt = np.ascontiguousarray((0.25 * Wk).T.reshape(C2T, 128, D)).astype(bf16)
    wvt = np.ascontiguousarray((0.25 * Wv).T.reshape(C2T, 128, D)).astype(bf16)
    # bk is softmax-invariant (constant per score row) and is dropped.
    # bv folds into the output bias because attention rows sum to one.
    bo_eff = bo + Wo @ bv
    wot = np.ascontiguousarray(
        np.concatenate([Wo.T, bo_eff[None, :]], axis=0)
    ).astype(bf16)
    bqv = np.ascontiguousarray(bq.reshape(D, 1))

    shared = {"wqt": wqt, "wkt": wkt, "wvt": wvt, "wot": wot, "bq": bqv}
    in_maps = []
    for b in range(B):
        m = dict(shared)
        m["x1"] = np.ascontiguousarray(x1[b].reshape(C1, HW))
        m["x2"] = np.ascontiguousarray(x2[b].reshape(C2, HW))
        in_maps.append(m)
    return in_maps


def run(inputs, trace=False, **trace_kwargs):
    from concourse.bass_utils import run_bass_kernel_spmd

    nc = _get_nc()
    in_maps = _prep_in_maps(**inputs)
    res = run_bass_kernel_spmd(
        nc, in_maps, list(range(B)), trace=trace, **trace_kwargs
    )
    out = np.stack([res.results[i]["out"] for i in range(B)])
    out = out.reshape(B, C1, H, W).astype(np.float32)
    return out, res


def kernel(**inputs) -> np.ndarray:
    out, _ = run(inputs, trace=False)
    return out



# revision 2
# speedup vs baseline: 1.1277x; 1.1277x over previous
"""Trainium2 Bass kernel for pooled cross-attention block (dense_transformer).

Reference computation per batch element b (B=8, one per NeuronCore):
  x2p = 2x2 mean-pool(x2)                      [512, 32, 32]
  Q = Wq @ x1  + bq                            [64, 4096]   (d-part layout)
  K = Wk @ x2p + bk                            [64, 1024]
  V = Wv @ x2p + bv                            [64, 1024]
  attn = softmax_m(K^T Q)                      [1024, 4096] (transposed)
  out  = V_aug^T attn                          [65, 4096]   (ones row -> r)
  y    = Wo_aug^T (out / r) ; result = x1 + y

Perf-relevant structure (vs the naive version):
  - score matmuls have contraction D=64 which leaves half the PE array idle
    AND keeps the PE HAM clock-gate cold (measured: lone K=64 matmuls never
    reach 2.4 GHz).  Fix: row-packed pairs via tile_position (0,0)/(64,0)
    with K and Q duplicated across both partition halves - two score tiles
    per PE slot at full clock.
  - exp runs on ACT as [128,1024] instructions spanning 2 PSUM banks
    (amortizes the ~311-cycle instruction overhead).
  - softmax denominator: r rides as the ones-row of V_aug through the U
    accumulation; 1/r = exp(-ln r) on ACT (same activation-table set as
    exp; DVE reciprocal is 8 cycles/element and 3.3us per chunk).
    rb = partition_broadcast(1/r) self-normalizes the ones row (r*1/r=1)
    so Wo_aug's bias row needs no special handling.
  - two-pass mi-streaming: pass 1 (K/V tiles 0-3 from the first half of
    x2) computes Q projections and stages exp(scores) in SBUF while the
    second half of x2 + x1 stream in; pass 2 does the rest + U/y/output.
    This hides most of the 23us x2 DMA behind compute.
  - biases: bk drops (softmax shift invariance); bv folds into
    bo' = bo + Wo@bv; bq added on the PSUM->SBUF copy of Q (DVE).
"""

import sys

for _p in ("/opt/trn_rl_repo",):
    if _p not in sys.path:
        sys.path.insert(0, _p)

import numpy as np

B, C1, C2, H, W, D = 8, 256, 512, 64, 64, 64
HW = H * W               # n (query) size: 4096
M = (H // 2) * (W // 2)  # kv size: 1024
NCH = 512                # n-chunk (one fp32 PSUM bank)
NCHUNKS = HW // NCH      # 8
C1T = C1 // 128          # 2
C2T = C2 // 128          # 4
MT = M // 128            # 8

_CACHE = {}


def _build():
    import concourse.bass as bass
    import concourse.tile as tile
    from concourse import bacc, mybir

    dt = mybir.dt
    f32, bf16, f32r = dt.float32, dt.bfloat16, dt.float32r
    Exp = mybir.ActivationFunctionType.Exp
    Ln = mybir.ActivationFunctionType.Ln

    nc = bacc.Bacc(
        "TRN2", target_bir_lowering=False, debug=False, num_devices=8
    )
    x1 = nc.dram_tensor("x1", [C1, HW], f32r, kind="ExternalInput").ap()
    x2 = nc.dram_tensor("x2", [C2, HW], f32, kind="ExternalInput").ap()
    wqt2 = nc.dram_tensor("wqt2", [C1T, 128, 128], f32r, kind="ExternalInput").ap()
    wkt2 = nc.dram_tensor("wkt2", [C2T, 128, 128], bf16, kind="ExternalInput").ap()
    wvt = nc.dram_tensor("wvt", [C2T, 128, D], bf16, kind="ExternalInput").ap()
    wot = nc.dram_tensor("wot", [D + 1, C1], bf16, kind="ExternalInput").ap()
    bq2 = nc.dram_tensor("bq2", [128, 1], f32, kind="ExternalInput").ap()
    out = nc.dram_tensor("out", [C1, HW], f32, kind="ExternalOutput").ap()

    from contextlib import ExitStack

    with tile.TileContext(nc) as tc, ExitStack() as ctx:
        pool = lambda name, bufs, **kw: ctx.enter_context(
            tc.tile_pool(name=name, bufs=bufs, **kw)
        )
        consts = pool("consts", 1)
        x2p = pool("x2p", 3)
        poolp = pool("poolp", 2)
        sbfp = pool("sbfp", 2)
        kvsb = pool("kvsb", 1)
        x1p = pool("x1p", 1)
        qsbp = pool("qsbp", 1)
        es1p = pool("es1p", 1)
        es2p = pool("es2p", 3)
        rallp = pool("rallp", 1)
        rinvp = pool("rinvp", 2)
        rbp = pool("rbp", 2)
        onp = pool("onp", 2)
        youtp = pool("youtp", 4)
        ps_sp = pool("ps_sp", 2, space="PSUM")   # [128,1024] pairs (2 banks)
        ps_u = pool("ps_u", 2, space="PSUM")     # [65,512] u / [128,256] v
        ps_qy = pool("ps_qy", 2, space="PSUM")   # [128,512] q / y

        # ---- constants -------------------------------------------------
        wqt2_sb = consts.tile([128, C1T, 128], f32r, tag="wqt2")
        for t in range(C1T):
            nc.sync.dma_start(out=wqt2_sb[:, t, :], in_=wqt2[t])
        wkt2_sb = consts.tile([128, C2T, 128], bf16, tag="wkt2")
        wvt_sb = consts.tile([128, C2T, D], bf16, tag="wvt")
        for ci in range(C2T):
            nc.sync.dma_start(out=wkt2_sb[:, ci, :], in_=wkt2[ci])
            nc.sync.dma_start(out=wvt_sb[:, ci, :], in_=wvt[ci])
        wot_sb = consts.tile([D + 1, C1], bf16, tag="wot")
        nc.sync.dma_start(out=wot_sb, in_=wot)
        bq2_sb = consts.tile([128, 1], f32, tag="bq2")
        nc.sync.dma_start(out=bq2_sb, in_=bq2)

        # persistent SBUF state
        x1_all = x1p.tile([128, C1T, HW], f32r, tag="x1all")
        q2_all = qsbp.tile([128, NCHUNKS, NCH], bf16, tag="q2all")
        es1_all = es1p.tile([128, NCHUNKS, 4 * NCH], bf16, tag="es1all")
        k2_sb = kvsb.tile([128, M], bf16, tag="k2sb")
        v_aug = kvsb.tile([128, MT, D + 1], bf16, tag="vaug")
        r_all = rallp.tile([1, HW], f32, tag="rall")
        nc.vector.memset(v_aug[:, :, D], 1.0)

        # ---- phase A: pool x2, project K (dup rows) and V^T ------------
        half = HW // 2
        for hi in range(2):
            k_ps = ps_sp.tile([128, NCH], f32, tag="sp", name=f"k_ps{hi}")
            v_ps = ps_u.tile(
                [128, MT // 2, D], f32, tag="u", name=f"v_ps{hi}"
            )
            for ci in range(C2T):
                if hi == 1:
                    # interleave one x1 chunk-pair load between x2 tiles
                    g = ci
                    for t in range(C1T):
                        nc.sync.dma_start(
                            out=x1_all[:, t, g * 1024:(g + 1) * 1024],
                            in_=x1[t * 128:(t + 1) * 128, g * 1024:(g + 1) * 1024],
                        )
                x2t = x2p.tile([128, half], f32, tag="x2t", name="x2t")
                nc.sync.dma_start(
                    out=x2t,
                    in_=x2[ci * 128:(ci + 1) * 128, hi * half:(hi + 1) * half],
                )
                x2v = x2t.rearrange("p (h w2 two) -> p h w2 two", w2=W // 2, two=2)
                t1 = poolp.tile([128, H // 2, W // 2], f32, tag="t1", name="t1")
                nc.vector.tensor_add(t1, x2v[:, :, :, 0], x2v[:, :, :, 1])
                t1v = t1.rearrange("p (h2 two) w2 -> p h2 two w2", two=2)
                s_bf = sbfp.tile([128, NCH], bf16, tag="sbf", name="s_bf")
                s3 = s_bf.rearrange("p (h2 w2) -> p h2 w2", h2=H // 4)
                nc.vector.tensor_add(s3, t1v[:, :, 0, :], t1v[:, :, 1, :])
                nc.tensor.matmul(
                    k_ps,
                    lhsT=wkt2_sb[:, ci, :],
                    rhs=s_bf,
                    start=(ci == 0),
                    stop=(ci == C2T - 1),
                )
                for mj in range(MT // 2):
                    nc.tensor.matmul(
                        v_ps[:, mj, :],
                        lhsT=s_bf[:, mj * 128:(mj + 1) * 128],
                        rhs=wvt_sb[:, ci, :],
                        start=(ci == 0),
                        stop=(ci == C2T - 1),
                    )
            nc.vector.tensor_copy(k2_sb[:, hi * NCH:(hi + 1) * NCH], k_ps)
            for mj in range(MT // 2):
                nc.vector.tensor_copy(
                    v_aug[:, hi * (MT // 2) + mj, 0:D], v_ps[:, mj, :]
                )

        def spair(nj, pr, dest):
            """Row-packed score pair: mi=(2pr, 2pr+1) for chunk nj."""
            miA, miB = 2 * pr, 2 * pr + 1
            sp = ps_sp.tile([128, 2 * NCH], f32, tag="sp", name=f"sp{nj}_{pr}")
            nc.tensor.matmul(
                sp[:, 0:NCH],
                lhsT=k2_sb[0:64, miA * 128:(miA + 1) * 128],
                rhs=q2_all[0:64, nj, :],
                start=True,
                stop=True,
                tile_position=(0, 0),
            )
            nc.tensor.matmul(
                sp[:, NCH:2 * NCH],
                lhsT=k2_sb[64:128, miB * 128:(miB + 1) * 128],
                rhs=q2_all[64:128, nj, :],
                start=True,
                stop=True,
                tile_position=(64, 0),
            )
            nc.scalar.activation(dest, sp, Exp)

        # ---- pass 1: Q projections + exp(scores) for mi 0..3 -----------
        for nj in range(NCHUNKS):
            nsl = slice(nj * NCH, (nj + 1) * NCH)
            q_ps = ps_qy.tile([128, NCH], f32, tag="qy", name=f"q_ps{nj}")
            for t in range(C1T):
                nc.tensor.matmul(
                    q_ps,
                    lhsT=wqt2_sb[:, t, :],
                    rhs=x1_all[:, t, nsl],
                    start=(t == 0),
                    stop=(t == C1T - 1),
                )
            nc.vector.tensor_scalar_add(q2_all[:, nj, :], q_ps, bq2_sb)
            for pr in range(2):
                spair(nj, pr, es1_all[:, nj, pr * 2 * NCH:(pr + 1) * 2 * NCH])

        # ---- pass 2: remaining scores, U, normalize, y, output ---------
        for njp in range(NCHUNKS // 2):
            u_tiles = []
            for sub in range(2):
                nj = 2 * njp + sub
                nsl = slice(nj * NCH, (nj + 1) * NCH)
                es2_tiles = []
                for pr in range(2, 4):
                    es2 = es2p.tile(
                        [128, 2 * NCH], bf16, tag="es2", name=f"es2_{nj}_{pr}"
                    )
                    spair(nj, pr, es2)
                    es2_tiles.append(es2)
                u_ps = ps_u.tile([D + 1, NCH], f32, tag="u", name=f"u{nj}")
                for mi in range(4):
                    nc.tensor.matmul(
                        u_ps,
                        lhsT=v_aug[:, mi, :],
                        rhs=es1_all[:, nj, mi * NCH:(mi + 1) * NCH],
                        start=(mi == 0),
                        stop=False,
                    )
                for pr in range(2):
                    for h in range(2):
                        mi = 4 + 2 * pr + h
                        nc.tensor.matmul(
                            u_ps,
                            lhsT=v_aug[:, mi, :],
                            rhs=es2_tiles[pr][:, h * NCH:(h + 1) * NCH],
                            start=False,
                            stop=(mi == MT - 1),
                        )
                nc.vector.tensor_copy(r_all[0:1, nsl], u_ps[D:D + 1, :])
                u_tiles.append(u_ps)
            # shared 1/r for the chunk pair: exp(-ln r) on ACT
            psl = slice(2 * njp * NCH, (2 * njp + 2) * NCH)
            lnr = rinvp.tile([1, 2 * NCH], f32, tag="lnr", name=f"lnr{njp}")
            nc.scalar.activation(lnr, r_all[0:1, psl], Ln)
            rinv = rinvp.tile([1, 2 * NCH], f32, tag="rinv", name=f"rinv{njp}")
            nc.scalar.activation(rinv, lnr, Exp, scale=-1.0)
            rb = rbp.tile([D + 1, 2 * NCH], f32, tag="rb", name=f"rb{njp}")
            nc.gpsimd.partition_broadcast(rb, rinv)
            for sub in range(2):
                nj = 2 * njp + sub
                nsl = slice(nj * NCH, (nj + 1) * NCH)
                on = onp.tile([D + 1, NCH], bf16, tag="on", name=f"on{nj}")
                nc.vector.tensor_mul(
                    on, u_tiles[sub], rb[:, sub * NCH:(sub + 1) * NCH]
                )
                for t in range(C1T):
                    y_ps = ps_qy.tile([128, NCH], f32, tag="qy", name=f"y{nj}_{t}")
                    nc.tensor.matmul(
                        y_ps,
                        lhsT=wot_sb[:, t * 128:(t + 1) * 128],
                        rhs=on,
                        start=True,
                        stop=True,
                    )
                    yo = youtp.tile([128, NCH], f32, tag="yo", name=f"yo{nj}_{t}")
                    nc.vector.tensor_add(yo, x1_all[:, t, nsl].bitcast(f32), y_ps)
                    nc.sync.dma_start(out=out[t * 128:(t + 1) * 128, nsl], in_=yo)
    nc.compile()
    return nc


def _get_nc():
    if "nc" not in _CACHE:
        _CACHE["nc"] = _build()
    return _CACHE["nc"]


def _prep_in_maps(x1, x2, Wq, bq, Wk, bk, Wv, bv, Wo, bo):
    import ml_dtypes

    bf16 = ml_dtypes.bfloat16
    f32 = np.float32
    x1 = np.asarray(x1, f32)
    x2 = np.asarray(x2, f32)
    Wq = np.asarray(Wq, f32)
    Wk = np.asarray(Wk, f32)
    Wv = np.asarray(Wv, f32)
    Wo = np.asarray(Wo, f32)
    bq = np.asarray(bq, f32)
    bk = np.asarray(bk, f32)
    bv = np.asarray(bv, f32)
    bo = np.asarray(bo, f32)

    # Wq^T / Wk^T tiles with the output (d) dimension duplicated into both
    # partition halves, so Q and K come out of the PE pre-duplicated for
    # the row-packed score matmuls.
    wqt = Wq.T.reshape(C1T, 128, D)
    wqt2 = np.ascontiguousarray(np.concatenate([wqt, wqt], axis=2))
    wkt = (0.25 * Wk).T.reshape(C2T, 128, D)
    wkt2 = np.ascontiguousarray(np.concatenate([wkt, wkt], axis=2)).astype(bf16)
    wvt = np.ascontiguousarray((0.25 * Wv).T.reshape(C2T, 128, D)).astype(bf16)
    # bk is softmax-invariant (constant per score row) and is dropped.
    # bv folds into the output bias because attention rows sum to one.
    bo_eff = bo + Wo @ bv
    wot = np.ascontiguousarray(
        np.concatenate([Wo.T, bo_eff[None, :]], axis=0)
    ).astype(bf16)
    bq2 = np.ascontiguousarray(np.concatenate([bq, bq]).reshape(128, 1))

    shared = {"wqt2": wqt2, "wkt2": wkt2, "wvt": wvt, "wot": wot, "bq2": bq2}
    in_maps = []
    for b in range(B):
        m = dict(shared)
        m["x1"] = np.ascontiguousarray(x1[b].reshape(C1, HW))
        m["x2"] = np.ascontiguousarray(x2[b].reshape(C2, HW))
        in_maps.append(m)
    return in_maps


def run(inputs, trace=False, **trace_kwargs):
    from concourse.bass_utils import run_bass_kernel_spmd

    nc = _get_nc()
    in_maps = _prep_in_maps(**inputs)
    res = run_bass_kernel_spmd(
        nc, in_maps, list(range(B)), trace=trace, **trace_kwargs
    )
    out = np.stack([res.results[i]["out"] for i in range(B)])
    out = out.reshape(B, C1, H, W).astype(np.float32)
    return out, res


def kernel(**inputs) -> np.ndarray:
    out, _ = run(inputs, trace=False)
    return out


# revision 5
# speedup vs baseline: 1.1845x; 1.0504x over previous
"""Trainium2 Bass kernel for pooled cross-attention block (dense_transformer).

Reference computation per batch element b (B=8, one per NeuronCore):
  x2p = 2x2 mean-pool(x2)                      [512, 32, 32]
  Q = Wq @ x1  + bq                            [64, 4096]   (d-part layout)
  K = Wk @ x2p + bk                            [64, 1024]
  V = Wv @ x2p + bv                            [64, 1024]
  attn = softmax_m(K^T Q)                      [1024, 4096] (transposed)
  out  = V_aug^T attn                          [65, 4096]   (ones row -> r)
  y    = Wo_aug^T (out / r) ; result = x1 + y

Perf-relevant structure (vs the naive version):
  - score matmuls have contraction D=64 which leaves half the PE array idle
    AND keeps the PE HAM clock-gate cold (measured: lone K=64 matmuls never
    reach 2.4 GHz).  Fix: row-packed pairs via tile_position (0,0)/(64,0)
    with K and Q duplicated across both partition halves - two score tiles
    per PE slot at full clock.
  - exp runs on ACT as [128,1024] instructions spanning 2 PSUM banks
    (amortizes the ~311-cycle instruction overhead).
  - softmax denominator: r rides as the ones-row of V_aug through the U
    accumulation; 1/r = exp(-ln r) on ACT (same activation-table set as
    exp; DVE reciprocal is 8 cycles/element and 3.3us per chunk).
    rb = partition_broadcast(1/r) self-normalizes the ones row (r*1/r=1)
    so Wo_aug's bias row needs no special handling.
  - two-pass mi-streaming: pass 1 (K/V tiles 0-3 from the first half of
    x2) computes Q projections and stages exp(scores) in SBUF while the
    second half of x2 + x1 stream in; pass 2 does the rest + U/y/output.
    This hides most of the 23us x2 DMA behind compute.
  - biases: bk drops (softmax shift invariance); bv folds into
    bo' = bo + Wo@bv; bq added on the PSUM->SBUF copy of Q (DVE).
"""

import sys

for _p in ("/opt/trn_rl_repo",):
    if _p not in sys.path:
        sys.path.insert(0, _p)

import numpy as np

B, C1, C2, H, W, D = 8, 256, 512, 64, 64, 64
HW = H * W               # n (query) size: 4096
M = (H // 2) * (W // 2)  # kv size: 1024
NCH = 512                # n-chunk (one fp32 PSUM bank)
NCHUNKS = HW // NCH      # 8
C1T = C1 // 128          # 2
C2T = C2 // 128          # 4
MT = M // 128            # 8

_CACHE = {}


def _build():
    import concourse.bass as bass
    import concourse.tile as tile
    from concourse import bacc, mybir
    import concourse.hw_specs as hw_specs

    # Make the activation-table insertion pass resolve BOTH Exp and Ln to
    # the one table set that contains them together; otherwise every
    # Ln/Exp alternation emits an ACT_TABLE_LOAD (~1.3us ACT stall each,
    # and the PE idles long enough to re-throttle its clock).  The pass
    # picks the first set containing the function, so list the combined
    # set first.
    _orig_tables = hw_specs.get_activation_tables

    def _tables_pref(arch):
        d = _orig_tables(arch)
        pref = "natural_log_exp_and_others"
        if pref in d:
            nd = {pref: d[pref]}
            nd.update({k: v for k, v in d.items() if k != pref})
            return nd
        return d

    dt = mybir.dt
    f32, bf16, f32r = dt.float32, dt.bfloat16, dt.float32r
    Exp = mybir.ActivationFunctionType.Exp
    Ln = mybir.ActivationFunctionType.Ln

    hw_specs.get_activation_tables = _tables_pref
    bacc.get_activation_tables = _tables_pref

    nc = bacc.Bacc(
        "TRN2", target_bir_lowering=False, debug=False, num_devices=8
    )
    x1 = nc.dram_tensor("x1", [C1, HW], f32r, kind="ExternalInput").ap()
    x2 = nc.dram_tensor("x2", [C2, HW], f32, kind="ExternalInput").ap()
    wqt2 = nc.dram_tensor("wqt2", [C1T, 128, 128], f32r, kind="ExternalInput").ap()
    wkt2 = nc.dram_tensor("wkt2", [C2T, 128, 128], bf16, kind="ExternalInput").ap()
    wvt = nc.dram_tensor("wvt", [C2T, 128, D], bf16, kind="ExternalInput").ap()
    wot = nc.dram_tensor("wot", [D + 1, C1], bf16, kind="ExternalInput").ap()
    bq2 = nc.dram_tensor("bq2", [128, 1], f32, kind="ExternalInput").ap()
    out = nc.dram_tensor("out", [C1, HW], f32, kind="ExternalOutput").ap()

    from contextlib import ExitStack

    with tile.TileContext(nc) as tc, ExitStack() as ctx:
        pool = lambda name, bufs, **kw: ctx.enter_context(
            tc.tile_pool(name=name, bufs=bufs, **kw)
        )
        consts = pool("consts", 1)
        x2p = pool("x2p", 3)
        poolp = pool("poolp", 2)
        sbfp = pool("sbfp", 2)
        kvsb = pool("kvsb", 1)
        x1p = pool("x1p", 1)
        qsbp = pool("qsbp", 1)
        es1p = pool("es1p", 1)
        es2p = pool("es2p", 3)
        rallp = pool("rallp", 1)
        rinvp = pool("rinvp", 2)
        rbp = pool("rbp", 2)
        onp = pool("onp", 2)
        youtp = pool("youtp", 4)
        ps_sp = pool("ps_sp", 2, space="PSUM")   # [128,1024] pairs (2 banks)
        ps_u = pool("ps_u", 2, space="PSUM")     # [65,512] u / [128,256] v
        ps_qy = pool("ps_qy", 2, space="PSUM")   # [128,512] q / y

        # ---- constants -------------------------------------------------
        wqt2_sb = consts.tile([128, C1T, 128], f32r, tag="wqt2")
        for t in range(C1T):
            nc.sync.dma_start(out=wqt2_sb[:, t, :], in_=wqt2[t])
        wkt2_sb = consts.tile([128, C2T, 128], bf16, tag="wkt2")
        wvt_sb = consts.tile([128, C2T, D], bf16, tag="wvt")
        for ci in range(C2T):
            nc.sync.dma_start(out=wkt2_sb[:, ci, :], in_=wkt2[ci])
            nc.sync.dma_start(out=wvt_sb[:, ci, :], in_=wvt[ci])
        wot_sb = consts.tile([D + 1, C1], bf16, tag="wot")
        nc.sync.dma_start(out=wot_sb, in_=wot)
        bq2_sb = consts.tile([128, 1], f32, tag="bq2")
        nc.sync.dma_start(out=bq2_sb, in_=bq2)

        # persistent SBUF state
        x1_all = x1p.tile([128, C1T, HW], f32r, tag="x1all")
        q2_all = qsbp.tile([128, NCHUNKS, NCH], bf16, tag="q2all")
        es1_all = es1p.tile([128, NCHUNKS, 4 * NCH], bf16, tag="es1all")
        k2_sb = kvsb.tile([128, M], bf16, tag="k2sb")
        v_aug = kvsb.tile([128, MT, D + 1], bf16, tag="vaug")
        r_all = rallp.tile([1, HW], f32, tag="rall")
        nc.vector.memset(v_aug[:, :, D], 1.0)

        # ---- phase A: pool x2, project K (dup rows) and V^T ------------
        half = HW // 2
        for hi in range(2):
            k_ps = ps_sp.tile([128, NCH], f32, tag="sp", name=f"k_ps{hi}")
            v_ps = ps_u.tile(
                [128, MT // 2, D], f32, tag="u", name=f"v_ps{hi}"
            )
            for ci in range(C2T):
                if hi == 1:
                    # interleave one x1 chunk-pair load between x2 tiles
                    g = ci
                    for t in range(C1T):
                        nc.sync.dma_start(
                            out=x1_all[:, t, g * 1024:(g + 1) * 1024],
                            in_=x1[t * 128:(t + 1) * 128, g * 1024:(g + 1) * 1024],
                        )
                x2t = x2p.tile([128, half], f32, tag="x2t", name="x2t")
                nc.sync.dma_start(
                    out=x2t,
                    in_=x2[ci * 128:(ci + 1) * 128, hi * half:(hi + 1) * half],
                )
                x2v = x2t.rearrange("p (h w2 two) -> p h w2 two", w2=W // 2, two=2)
                t1 = poolp.tile([128, H // 2, W // 2], f32, tag="t1", name="t1")
                nc.vector.tensor_add(t1, x2v[:, :, :, 0], x2v[:, :, :, 1])
                t1v = t1.rearrange("p (h2 two) w2 -> p h2 two w2", two=2)
                s_bf = sbfp.tile([128, NCH], bf16, tag="sbf", name="s_bf")
                s3 = s_bf.rearrange("p (h2 w2) -> p h2 w2", h2=H // 4)
                nc.vector.tensor_add(s3, t1v[:, :, 0, :], t1v[:, :, 1, :])
                nc.tensor.matmul(
                    k_ps,
                    lhsT=wkt2_sb[:, ci, :],
                    rhs=s_bf,
                    start=(ci == 0),
                    stop=(ci == C2T - 1),
                )
                for mj in range(MT // 2):
                    nc.tensor.matmul(
                        v_ps[:, mj, :],
                        lhsT=s_bf[:, mj * 128:(mj + 1) * 128],
                        rhs=wvt_sb[:, ci, :],
                        start=(ci == 0),
                        stop=(ci == C2T - 1),
                    )
            nc.vector.tensor_copy(k2_sb[:, hi * NCH:(hi + 1) * NCH], k_ps)
            for mj in range(MT // 2):
                nc.vector.tensor_copy(
                    v_aug[:, hi * (MT // 2) + mj, 0:D], v_ps[:, mj, :]
                )

        def spair(nj, pr, dest):
            """Row-packed score pair: mi=(2pr, 2pr+1) for chunk nj."""
            miA, miB = 2 * pr, 2 * pr + 1
            sp = ps_sp.tile([128, 2 * NCH], f32, tag="sp", name=f"sp{nj}_{pr}")
            nc.tensor.matmul(
                sp[:, 0:NCH],
                lhsT=k2_sb[0:64, miA * 128:(miA + 1) * 128],
                rhs=q2_all[0:64, nj, :],
                start=True,
                stop=True,
                tile_position=(0, 0),
            )
            nc.tensor.matmul(
                sp[:, NCH:2 * NCH],
                lhsT=k2_sb[64:128, miB * 128:(miB + 1) * 128],
                rhs=q2_all[64:128, nj, :],
                start=True,
                stop=True,
                tile_position=(64, 0),
            )
            nc.scalar.activation(dest, sp, Exp)

        # ---- pass 1: Q projections + exp(scores) for mi 0..3 -----------
        for nj in range(NCHUNKS):
            nsl = slice(nj * NCH, (nj + 1) * NCH)
            q_ps = ps_qy.tile([128, NCH], f32, tag="qy", name=f"q_ps{nj}")
            for t in range(C1T):
                nc.tensor.matmul(
                    q_ps,
                    lhsT=wqt2_sb[:, t, :],
                    rhs=x1_all[:, t, nsl],
                    start=(t == 0),
                    stop=(t == C1T - 1),
                )
            nc.vector.tensor_scalar_add(q2_all[:, nj, :], q_ps, bq2_sb)
            for pr in range(2):
                spair(nj, pr, es1_all[:, nj, pr * 2 * NCH:(pr + 1) * 2 * NCH])

        # ---- pass 2: remaining scores, U, normalize, y, output ---------
        for njp in range(NCHUNKS // 2):
            u_tiles = []
            for sub in range(2):
                nj = 2 * njp + sub
                nsl = slice(nj * NCH, (nj + 1) * NCH)
                es2_tiles = []
                for pr in range(2, 4):
                    es2 = es2p.tile(
                        [128, 2 * NCH], bf16, tag="es2", name=f"es2_{nj}_{pr}"
                    )
                    spair(nj, pr, es2)
                    es2_tiles.append(es2)
                u_ps = ps_u.tile([D + 1, NCH], f32, tag="u", name=f"u{nj}")
                for mi in range(4):
                    nc.tensor.matmul(
                        u_ps,
                        lhsT=v_aug[:, mi, :],
                        rhs=es1_all[:, nj, mi * NCH:(mi + 1) * NCH],
                        start=(mi == 0),
                        stop=False,
                    )
                for pr in range(2):
                    for h in range(2):
                        mi = 4 + 2 * pr + h
                        nc.tensor.matmul(
                            u_ps,
                            lhsT=v_aug[:, mi, :],
                            rhs=es2_tiles[pr][:, h * NCH:(h + 1) * NCH],
                            start=False,
                            stop=(mi == MT - 1),
                        )
                nc.vector.tensor_copy(r_all[0:1, nsl], u_ps[D:D + 1, :])
                u_tiles.append(u_ps)
            # shared 1/r for the chunk pair: exp(-ln r) on ACT
            psl = slice(2 * njp * NCH, (2 * njp + 2) * NCH)
            lnr = rinvp.tile([1, 2 * NCH], f32, tag="lnr", name=f"lnr{njp}")
            nc.scalar.activation(lnr, r_all[0:1, psl], Ln)
            rinv = rinvp.tile([1, 2 * NCH], f32, tag="rinv", name=f"rinv{njp}")
            nc.scalar.activation(rinv, lnr, Exp, scale=-1.0)
            rb = rbp.tile([D + 1, 2 * NCH], f32, tag="rb", name=f"rb{njp}")
            nc.gpsimd.partition_broadcast(rb, rinv)
            for sub in range(2):
                nj = 2 * njp + sub
                nsl = slice(nj * NCH, (nj + 1) * NCH)
                on = onp.tile([D + 1, NCH], bf16, tag="on", name=f"on{nj}")
                nc.vector.tensor_mul(
                    on, u_tiles[sub], rb[:, sub * NCH:(sub + 1) * NCH]
                )
                for t in range(C1T):
                    y_ps = ps_qy.tile([128, NCH], f32, tag="qy", name=f"y{nj}_{t}")
                    nc.tensor.matmul(
                        y_ps,
                        lhsT=wot_sb[:, t * 128:(t + 1) * 128],
                        rhs=on,
                        start=True,
                        stop=True,
                    )
                    yo = youtp.tile([128, NCH], f32, tag="yo", name=f"yo{nj}_{t}")
                    nc.vector.tensor_add(yo, x1_all[:, t, nsl].bitcast(f32), y_ps)
                    nc.sync.dma_start(out=out[t * 128:(t + 1) * 128, nsl], in_=yo)
    try:
        nc.compile()
    finally:
        hw_specs.get_activation_tables = _orig_tables
        bacc.get_activation_tables = _orig_tables
    return nc


def _get_nc():
    if "nc" not in _CACHE:
        _CACHE["nc"] = _build()
    return _CACHE["nc"]


def _prep_in_maps(x1, x2, Wq, bq, Wk, bk, Wv, bv, Wo, bo):
    import ml_dtypes

    bf16 = ml_dtypes.bfloat16
    f32 = np.float32
    x1 = np.asarray(x1, f32)
    x2 = np.asarray(x2, f32)
    Wq = np.asarray(Wq, f32)
    Wk = np.asarray(Wk, f32)
    Wv = np.asarray(Wv, f32)
    Wo = np.asarray(Wo, f32)
    bq = np.asarray(bq, f32)
    bk = np.asarray(bk, f32)
    bv = np.asarray(bv, f32)
    bo = np.asarray(bo, f32)

    # Wq^T / Wk^T tiles with the output (d) dimension duplicated into both
    # partition halves, so Q and K come out of the PE pre-duplicated for
    # the row-packed score matmuls.
    wqt = Wq.T.reshape(C1T, 128, D)
    wqt2 = np.ascontiguousarray(np.concatenate([wqt, wqt], axis=2))
    wkt = (0.25 * Wk).T.reshape(C2T, 128, D)
    wkt2 = np.ascontiguousarray(np.concatenate([wkt, wkt], axis=2)).astype(bf16)
    wvt = np.ascontiguousarray((0.25 * Wv).T.reshape(C2T, 128, D)).astype(bf16)
    # bk is softmax-invariant (constant per score row) and is dropped.
    # bv folds into the output bias because attention rows sum to one.
    bo_eff = bo + Wo @ bv
    wot = np.ascontiguousarray(
        np.concatenate([Wo.T, bo_eff[None, :]], axis=0)
    ).astype(bf16)
    bq2 = np.ascontiguousarray(np.concatenate([bq, bq]).reshape(128, 1))

    shared = {"wqt2": wqt2, "wkt2": wkt2, "wvt": wvt, "wot": wot, "bq2": bq2}
    in_maps = []
    for b in range(B):
        m = dict(shared)
        m["x1"] = np.ascontiguousarray(x1[b].reshape(C1, HW))
        m["x2"] = np.ascontiguousarray(x2[b].reshape(C2, HW))
        in_maps.append(m)
    return in_maps


def run(inputs, trace=False, **trace_kwargs):
    from concourse.bass_utils import run_bass_kernel_spmd

    nc = _get_nc()
    in_maps = _prep_in_maps(**inputs)
    res = run_bass_kernel_spmd(
        nc, in_maps, list(range(B)), trace=trace, **trace_kwargs
    )
    out = np.stack([res.results[i]["out"] for i in range(B)])
    out = out.reshape(B, C1, H, W).astype(np.float32)
    return out, res


def kernel(**inputs) -> np.ndarray:
    out, _ = run(inputs, trace=False)
    return out


# revision 32
# speedup vs baseline: 1.2372x; 1.0445x over previous
"""Trainium2 Bass kernel for pooled cross-attention block (dense_transformer).

Reference computation per batch element b (B=8, one per NeuronCore):
  x2p = 2x2 mean-pool(x2)                      [512, 32, 32]
  Q = Wq @ x1  + bq                            [64, 4096]   (d-part layout)
  K = Wk @ x2p + bk                            [64, 1024]
  V = Wv @ x2p + bv                            [64, 1024]
  attn = softmax_m(K^T Q)                      [1024, 4096] (transposed)
  out  = V_aug^T attn                          [65, 4096]   (ones row -> r)
  y    = Wo_aug^T (out / r) ; result = x1 + y

Perf-relevant structure (vs the naive version):
  - score matmuls have contraction D=64 which leaves half the PE array idle
    AND keeps the PE HAM clock-gate cold (measured: lone K=64 matmuls never
    reach 2.4 GHz).  Fix: row-packed pairs via tile_position (0,0)/(64,0)
    with K and Q duplicated across both partition halves - two score tiles
    per PE slot at full clock.
  - exp runs on ACT as [128,1024] instructions spanning 2 PSUM banks
    (amortizes the ~311-cycle instruction overhead).
  - softmax denominator: r rides as the ones-row of V_aug through the U
    accumulation; 1/r = exp(-ln r) on ACT (same activation-table set as
    exp; DVE reciprocal is 8 cycles/element and 3.3us per chunk).
    rb = partition_broadcast(1/r) self-normalizes the ones row (r*1/r=1)
    so Wo_aug's bias row needs no special handling.
  - two-pass mi-streaming: pass 1 (K/V tiles 0-3 from the first half of
    x2) computes Q projections and stages exp(scores) in SBUF while the
    second half of x2 + x1 stream in; pass 2 does the rest + U/y/output.
    This hides most of the 23us x2 DMA behind compute.
  - biases: bk drops (softmax shift invariance); bv folds into
    bo' = bo + Wo@bv; bq added on the PSUM->SBUF copy of Q (DVE).
"""

import sys

for _p in ("/opt/trn_rl_repo",):
    if _p not in sys.path:
        sys.path.insert(0, _p)

import numpy as np

B, C1, C2, H, W, D = 8, 256, 512, 64, 64, 64
HW = H * W               # n (query) size: 4096
M = (H // 2) * (W // 2)  # kv size: 1024
NCH = 512                # n-chunk (one fp32 PSUM bank)
NCHUNKS = HW // NCH      # 8
C1T = C1 // 128          # 2
C2T = C2 // 128          # 4
MT = M // 128            # 8

_CACHE = {}


def _build():
    import concourse.bass as bass
    import concourse.tile as tile
    from concourse import bacc, mybir
    import concourse.hw_specs as hw_specs

    # Make the activation-table insertion pass resolve BOTH Exp and Ln to
    # the one table set that contains them together; otherwise every
    # Ln/Exp alternation emits an ACT_TABLE_LOAD (~1.3us ACT stall each,
    # and the PE idles long enough to re-throttle its clock).  The pass
    # picks the first set containing the function, so list the combined
    # set first.
    _orig_tables = hw_specs.get_activation_tables

    def _tables_pref(arch):
        d = _orig_tables(arch)
        pref = "natural_log_exp_and_others"
        if pref in d:
            nd = {pref: d[pref]}
            nd.update({k: v for k, v in d.items() if k != pref})
            return nd
        return d

    dt = mybir.dt
    f32, bf16, f32r = dt.float32, dt.bfloat16, dt.float32r
    Exp = mybir.ActivationFunctionType.Exp
    Ln = mybir.ActivationFunctionType.Ln

    hw_specs.get_activation_tables = _tables_pref
    bacc.get_activation_tables = _tables_pref

    nc = bacc.Bacc(
        "TRN2", target_bir_lowering=False, debug=False, num_devices=8
    )
    x1 = nc.dram_tensor("x1", [C1, HW], f32r, kind="ExternalInput").ap()
    x2 = nc.dram_tensor("x2", [C2, HW], f32, kind="ExternalInput").ap()
    wqt2 = nc.dram_tensor("wqt2", [C1T, 128, 128], f32r, kind="ExternalInput").ap()
    wkt2 = nc.dram_tensor("wkt2", [C2T, 128, 128], bf16, kind="ExternalInput").ap()
    wvt = nc.dram_tensor("wvt", [C2T, 128, D], bf16, kind="ExternalInput").ap()
    wot = nc.dram_tensor("wot", [D + 1, C1], bf16, kind="ExternalInput").ap()
    bq2 = nc.dram_tensor("bq2", [128, 1], f32, kind="ExternalInput").ap()
    out = nc.dram_tensor("out", [C1, HW], f32, kind="ExternalOutput").ap()

    from contextlib import ExitStack

    with tile.TileContext(nc) as tc, ExitStack() as ctx:
        pool = lambda name, bufs, **kw: ctx.enter_context(
            tc.tile_pool(name=name, bufs=bufs, **kw)
        )
        consts = pool("consts", 1)
        x2p = pool("x2p", 5)
        poolp = pool("poolp", 3)
        sbfp = pool("sbfp", 3)
        kvsb = pool("kvsb", 1)
        x1p = pool("x1p", 1)
        qsbp = pool("qsbp", 1)
        es1p = pool("es1p", 1)
        es2p = pool("es2p", 4)
        rallp = pool("rallp", 2)
        rinvp = pool("rinvp", 3)
        rbp = pool("rbp", 3)
        onp = pool("onp", 3)
        youtp = pool("youtp", 6)
        # 8 PSUM banks total: score-pair 1x[128,1024]=2, u 4x[65,512]=4,
        # q/y/k 2x[128,512]=2.  Four u slots let TWO subsequent chunks'
        # U accumulations proceed while the current pair's normalize chain
        # (Ln/Exp/broadcast/mul) drains; the score pipeline tolerates a
        # single slot because exp immediately consumes each pair.  q/y/k
        # tolerates two: k_ps(h0) evacuates before the first Q projection
        # needs a slot, and pass-1 Q serialization hides under the DMA
        # pace.
        ps_sp = pool("ps_sp", 1, space="PSUM")
        ps_u = pool("ps_u", 4, space="PSUM")
        ps_qy = pool("ps_qy", 2, space="PSUM")

        # ---- input DMA head --------------------------------------------
        # Issue order tuned for earliest useful data: first x2 tile (so
        # pooling starts ASAP), then the phase-A weights, the rest of the
        # first x2 half, then the pass-1 weights.  wot (pass-2 only) goes
        # at the end of the h1 stream.
        wqt2_sb = consts.tile([128, C1T, 128], f32r, tag="wqt2")
        wkt2_sb = consts.tile([128, C2T, 128], bf16, tag="wkt2")
        wvt_sb = consts.tile([128, C2T, D], bf16, tag="wvt")
        wot_sb = consts.tile([D + 1, C1], bf16, tag="wot")
        bq2_sb = consts.tile([128, 1], f32, tag="bq2")
        half = HW // 2
        x2_h0 = []
        for ci in range(C2T):
            x2t = x2p.tile([128, half], f32, tag="x2t", name="x2t")
            nc.sync.dma_start(
                out=x2t, in_=x2[ci * 128:(ci + 1) * 128, 0:half]
            )
            x2_h0.append(x2t)
            if ci == 0:
                for cj in range(C2T):
                    nc.sync.dma_start(out=wkt2_sb[:, cj, :], in_=wkt2[cj])
                    nc.sync.dma_start(out=wvt_sb[:, cj, :], in_=wvt[cj])
        for t in range(C1T):
            nc.sync.dma_start(out=wqt2_sb[:, t, :], in_=wqt2[t])
        nc.sync.dma_start(out=bq2_sb, in_=bq2)

        # persistent SBUF state
        x1_all = x1p.tile([128, C1T, HW], f32r, tag="x1all")
        q2_all = qsbp.tile([128, NCHUNKS, NCH], bf16, tag="q2all")
        es1_all = es1p.tile([128, NCHUNKS, 4 * NCH], bf16, tag="es1all")
        k2_sb = kvsb.tile([128, M], bf16, tag="k2sb")
        v_aug = kvsb.tile([128, MT, D + 1], bf16, tag="vaug")
        nc.vector.memset(v_aug[:, :, D], 1.0)
        # first x1 chunk-pair ahead of the x2 second half so pass 1's
        # first Q projection isn't gated on the x2 tail
        for t in range(C1T):
            nc.sync.dma_start(
                out=x1_all[:, t, 0:1024],
                in_=x1[t * 128:(t + 1) * 128, 0:1024],
            )

        # ---- phase A: pool x2, project K (dup rows) and V^T ------------
        # k_ps uses the q/y PSUM tag: the second-half k accumulator stays
        # live for the whole x2 tail and must NOT occupy a score-pair slot
        # (that serializes all of pass 1 through one PSUM buffer).
        for hi in range(2):
            k_ps = ps_qy.tile([128, NCH], f32, tag="qy", name=f"k_ps{hi}")
            v_ps = ps_u.tile(
                [128, MT // 2, D], f32, tag="u", name=f"v_ps{hi}"
            )
            for ci in range(C2T):
                if hi == 0:
                    x2t = x2_h0[ci]
                else:
                    # interleave one x1 chunk-pair load between x2 tiles
                    # (g0 was loaded up front)
                    g = ci + 1
                    if g < 4:
                        for t in range(C1T):
                            nc.sync.dma_start(
                                out=x1_all[:, t, g * 1024:(g + 1) * 1024],
                                in_=x1[t * 128:(t + 1) * 128, g * 1024:(g + 1) * 1024],
                            )
                    x2t = x2p.tile([128, half], f32, tag="x2t", name="x2t")
                    nc.sync.dma_start(
                        out=x2t,
                        in_=x2[ci * 128:(ci + 1) * 128, hi * half:(hi + 1) * half],
                    )
                    if ci == C2T - 1:
                        nc.sync.dma_start(out=wot_sb, in_=wot)
                x2v = x2t.rearrange("p (h w2 two) -> p h w2 two", w2=W // 2, two=2)
                t1 = poolp.tile([128, H // 2, W // 2], f32, tag="t1", name="t1")
                nc.vector.tensor_add(t1, x2v[:, :, :, 0], x2v[:, :, :, 1])
                t1v = t1.rearrange("p (h2 two) w2 -> p h2 two w2", two=2)
                s_bf = sbfp.tile([128, NCH], bf16, tag="sbf", name="s_bf")
                s3 = s_bf.rearrange("p (h2 w2) -> p h2 w2", h2=H // 4)
                nc.vector.tensor_add(s3, t1v[:, :, 0, :], t1v[:, :, 1, :])
                nc.tensor.matmul(
                    k_ps,
                    lhsT=wkt2_sb[:, ci, :],
                    rhs=s_bf,
                    start=(ci == 0),
                    stop=(ci == C2T - 1),
                )
                for mj in range(MT // 2):
                    nc.tensor.matmul(
                        v_ps[:, mj, :],
                        lhsT=s_bf[:, mj * 128:(mj + 1) * 128],
                        rhs=wvt_sb[:, ci, :],
                        start=(ci == 0),
                        stop=(ci == C2T - 1),
                    )
            nc.vector.tensor_copy(k2_sb[:, hi * NCH:(hi + 1) * NCH], k_ps)
            for mj in range(MT // 2):
                nc.vector.tensor_copy(
                    v_aug[:, hi * (MT // 2) + mj, 0:D], v_ps[:, mj, :]
                )

        def spair(nj, pr, dest):
            """Row-packed score pair: mi=(2pr, 2pr+1) for chunk nj."""
            miA, miB = 2 * pr, 2 * pr + 1
            sp = ps_sp.tile([128, 2 * NCH], f32, tag="sp", name=f"sp{nj}_{pr}")
            nc.tensor.matmul(
                sp[:, 0:NCH],
                lhsT=k2_sb[0:64, miA * 128:(miA + 1) * 128],
                rhs=q2_all[0:64, nj, :],
                start=True,
                stop=True,
                tile_position=(0, 0),
            )
            nc.tensor.matmul(
                sp[:, NCH:2 * NCH],
                lhsT=k2_sb[64:128, miB * 128:(miB + 1) * 128],
                rhs=q2_all[64:128, nj, :],
                start=True,
                stop=True,
                tile_position=(64, 0),
            )
            nc.scalar.activation(dest, sp, Exp)

        # ---- pass 1: Q projections + exp(scores) for mi 0..3 -----------
        for nj in range(NCHUNKS):
            nsl = slice(nj * NCH, (nj + 1) * NCH)
            q_ps = ps_qy.tile([128, NCH], f32, tag="qy", name=f"q_ps{nj}")
            for t in range(C1T):
                nc.tensor.matmul(
                    q_ps,
                    lhsT=wqt2_sb[:, t, :],
                    rhs=x1_all[:, t, nsl],
                    start=(t == 0),
                    stop=(t == C1T - 1),
                )
            nc.vector.tensor_scalar_add(q2_all[:, nj, :], q_ps, bq2_sb)
            for pr in range(2):
                spair(nj, pr, es1_all[:, nj, pr * 2 * NCH:(pr + 1) * 2 * NCH])

        # ---- pass 2: remaining scores, U, normalize, y, output ---------
        for njp in range(NCHUNKS // 2):
            # pair's r values live at partitions 0 and 32 (the only legal
            # DVE output partition bases); activation/Newton cost scales
            # with the free dim only, so [64,512] costs the same as [2,512]
            r_pair = rallp.tile([64, NCH], f32, tag="rpair", name=f"rp{njp}")
            u_tiles = []
            for sub in range(2):
                nj = 2 * njp + sub
                nsl = slice(nj * NCH, (nj + 1) * NCH)
                es2_tiles = []
                for pr in range(2, 4):
                    es2 = es2p.tile(
                        [128, 2 * NCH], bf16, tag="es2", name=f"es2_{nj}_{pr}"
                    )
                    spair(nj, pr, es2)
                    es2_tiles.append(es2)
                u_ps = ps_u.tile([D + 1, NCH], f32, tag="u", name=f"u{nj}")
                for mi in range(4):
                    nc.tensor.matmul(
                        u_ps,
                        lhsT=v_aug[:, mi, :],
                        rhs=es1_all[:, nj, mi * NCH:(mi + 1) * NCH],
                        start=(mi == 0),
                        stop=False,
                    )
                for pr in range(2):
                    for h in range(2):
                        mi = 4 + 2 * pr + h
                        nc.tensor.matmul(
                            u_ps,
                            lhsT=v_aug[:, mi, :],
                            rhs=es2_tiles[pr][:, h * NCH:(h + 1) * NCH],
                            start=False,
                            stop=(mi == MT - 1),
                        )
                nc.vector.tensor_copy(
                    r_pair[32 * sub:32 * sub + 1, :], u_ps[D:D + 1, :]
                )
                u_tiles.append(u_ps)
            # 1/r for the chunk pair: exp(-ln r) on ACT (same activation
            # table set as the score exps, so no table switches).  Keeping
            # this off the DVE keeps the pair tail short - the u PSUM
            # slots are released by the on-muls below, and the next pair's
            # U accumulation blocks on them.  Partitions other than 0 and
            # 32 carry garbage and are never read.
            lnr = rinvp.tile([64, NCH], f32, tag="lnr", name=f"lnr{njp}")
            nc.scalar.activation(lnr, r_pair, Ln)
            rinv = rinvp.tile([64, NCH], f32, tag="rinv", name=f"ri{njp}")
            nc.scalar.activation(rinv, lnr, Exp, scale=-1.0)
            # partition_broadcast needs its source at the tile's partition 0
            # (base-32 slices read garbage - verified on HW), so stage the
            # second chunk's rinv down to a base-0 scratch first.
            rinvB = rinvp.tile([1, NCH], f32, tag="rinvB", name=f"riB{njp}")
            nc.vector.tensor_copy(rinvB, rinv[32:33, :])
            rsrc = [rinv[0:1, :], rinvB]
            for sub in range(2):
                nj = 2 * njp + sub
                nsl = slice(nj * NCH, (nj + 1) * NCH)
                rb = rbp.tile([D + 1, NCH], f32, tag="rb", name=f"rb{nj}")
                nc.gpsimd.partition_broadcast(rb, rsrc[sub])
                on = onp.tile([D + 1, NCH], bf16, tag="on", name=f"on{nj}")
                nc.vector.tensor_mul(on, u_tiles[sub], rb)
                for t in range(C1T):
                    y_ps = ps_qy.tile([128, NCH], f32, tag="qy", name=f"y{nj}_{t}")
                    nc.tensor.matmul(
                        y_ps,
                        lhsT=wot_sb[:, t * 128:(t + 1) * 128],
                        rhs=on,
                        start=True,
                        stop=True,
                    )
                    yo = youtp.tile([128, NCH], f32, tag="yo", name=f"yo{nj}_{t}")
                    nc.vector.tensor_add(yo, x1_all[:, t, nsl].bitcast(f32), y_ps)
                    nc.sync.dma_start(out=out[t * 128:(t + 1) * 128, nsl], in_=yo)
    try:
        nc.compile()
    finally:
        hw_specs.get_activation_tables = _orig_tables
        bacc.get_activation_tables = _orig_tables
    return nc


def _get_nc():
    if "nc" not in _CACHE:
        _CACHE["nc"] = _build()
    return _CACHE["nc"]


def _prep_in_maps(x1, x2, Wq, bq, Wk, bk, Wv, bv, Wo, bo):
    import ml_dtypes

    bf16 = ml_dtypes.bfloat16
    f32 = np.float32
    x1 = np.asarray(x1, f32)
    x2 = np.asarray(x2, f32)
    Wq = np.asarray(Wq, f32)
    Wk = np.asarray(Wk, f32)
    Wv = np.asarray(Wv, f32)
    Wo = np.asarray(Wo, f32)
    bq = np.asarray(bq, f32)
    bk = np.asarray(bk, f32)
    bv = np.asarray(bv, f32)
    bo = np.asarray(bo, f32)

    # Wq^T / Wk^T tiles with the output (d) dimension duplicated into both
    # partition halves, so Q and K come out of the PE pre-duplicated for
    # the row-packed score matmuls.
    wqt = Wq.T.reshape(C1T, 128, D)
    wqt2 = np.ascontiguousarray(np.concatenate([wqt, wqt], axis=2))
    wkt = (0.25 * Wk).T.reshape(C2T, 128, D)
    wkt2 = np.ascontiguousarray(np.concatenate([wkt, wkt], axis=2)).astype(bf16)
    wvt = np.ascontiguousarray((0.25 * Wv).T.reshape(C2T, 128, D)).astype(bf16)
    # bk is softmax-invariant (constant per score row) and is dropped.
    # bv folds into the output bias because attention rows sum to one.
    bo_eff = bo + Wo @ bv
    wot = np.ascontiguousarray(
        np.concatenate([Wo.T, bo_eff[None, :]], axis=0)
    ).astype(bf16)
    bq2 = np.ascontiguousarray(np.concatenate([bq, bq]).reshape(128, 1))

    shared = {"wqt2": wqt2, "wkt2": wkt2, "wvt": wvt, "wot": wot, "bq2": bq2}
    in_maps = []
    for b in range(B):
        m = dict(shared)
        m["x1"] = np.ascontiguousarray(x1[b].reshape(C1, HW))
        m["x2"] = np.ascontiguousarray(x2[b].reshape(C2, HW))
        in_maps.append(m)
    return in_maps


def run(inputs, trace=False, **trace_kwargs):
    from concourse.bass_utils import run_bass_kernel_spmd

    nc = _get_nc()
    in_maps = _prep_in_maps(**inputs)
    res = run_bass_kernel_spmd(
        nc, in_maps, list(range(B)), trace=trace, **trace_kwargs
    )
    out = np.stack([res.results[i]["out"] for i in range(B)])
    out = out.reshape(B, C1, H, W).astype(np.float32)
    return out, res


def kernel(**inputs) -> np.ndarray:
    out, _ = run(inputs, trace=False)
    return out
